# revision 17
# baseline (speedup 1.0000x reference)
"""Fused linear + cross-entropy loss (chunked logsumexp) on 8 NeuronCores.

Strategy: tensor-parallel over vocab. Each core holds a 4000-row shard of
head_weight, computes logits = h @ W_c^T for all 4096 tokens (fp8e4m3
DoubleRow matmuls by default; bf16 fallback when head_bias is nonzero),
and reduces sum(exp(logit)) per token on the ACT engine (exp with
accum_out; the pre-exp rescale for the fp8 weight scaling rides the
ACT's free scale operand). The target-logit term h[t] . W[label_t] is
host glue (0.4% of the FLOPs), as are the final log over 4096 values
and the weighted mean.

Startup tuning: Tile's dependency tracking is whole-tile, so the first
matmul group waits for ALL of h block 0 + weight chunk (0,0); DMA
bandwidth scales with per-partition run length (4KB runs ~300 GB/s,
2KB ~150, 1KB ~70), so sub-tile pieces don't land any earlier than
full tiles. Hence: h0 rides the sync hardware-DGE queue (ring up
~8.2us, lands ~9.9), w00 rides the scalar hardware-DGE queue (lands
~11.0), and ALL remaining input (w0_1..3, h blocks 1-7, w1_*) rides
the gpsimd software-DGE queue, which sustains ~296 GB/s with 4KB-run
full tiles and keeps the two hardware queues free. 44 junk warmup
matmuls (from a vector-engine-memset tile) bridge the tensor queue
from preamble end (~7.1us) to first data (~12.1us) and hold the HAM
clock-boost activity window: the full-clock grant needs ~3.3-4.5us of
CONTINUOUS PE activity, and any idle gap before the grant resets the
window (costs ~5-8us — measured; do not undershoot the warmup count).
The first four token tiles run as two 2-tile "mini" PSUM allocations
against chunk 0 only, followed by 3-chunk rest-steps.

Tail: the last two steps are split across both PSUM buffers so only a
~1.1us half-width ACT follows the final matmul, and hsums drains in
slices so the final output DMA is one 2KB packet. The extra
accumulator columns (minis + split-step trailing halves) are folded on
the host.
"""

import numpy as np
import ml_dtypes

T = 4096
D = 1024
V = 32000
NCORES = 8
VSH = V // NCORES        # 4000 vocab rows per core
CPH = VSH // 2           # 2000 vocab cols per half
TT = T // 128            # 32 token tiles
TBC = 512                # tokens per resident ht block
NTB = T // TBC           # 8 ht col blocks

W_SCALE = 32.0           # fp8 path: W is scaled by this before casting
USE_FP8 = True
NT0 = 4                  # leading token tiles processed as mini+rest steps
WARM_N = 44

_CACHE = {}


def _chunks(cols):
    """Split cols into matmul free-dim chunks (<=512, 16-aligned)."""
    out = []
    while cols > 0:
        c = min(cols, 512)
        out.append(c)
        cols -= c
    assert all(c % 16 == 0 for c in out)
    return out


def _build(kt, mode, t=T, vsh=VSH, d=D, warm_n=WARM_N, do_compile=True):
    """Build+compile the SPMD Bass program.

    kt: number of 128-deep k tiles (8, or 9 when a nonzero head_bias is
        folded in as an extra contraction row).
    mode: "bf16" (plain matmuls) or "fp8dr" (fp8e4m3 DoubleRow, kt even).
    """
    import concourse.bass as bass
    import concourse.mybir as mybir
    import concourse.tile as tile
    from concourse import bacc

    f32 = mybir.dt.float32
    bf16 = mybir.dt.bfloat16
    fp8 = mybir.dt.float8e4
    AF = mybir.ActivationFunctionType
    ALU = mybir.AluOpType

    fp8dr = mode == "fp8dr"
    mdt = fp8 if fp8dr else bf16
    act_scale = (1.0 / W_SCALE) if fp8dr else 1.0
    if fp8dr:
        assert kt % 2 == 0
    nk = kt // 2 if fp8dr else kt   # matmul contraction steps

    tt = t // 128
    tb = min(TBC // 128, tt)   # token tiles per ht block
    ntb = tt // tb
    cph = vsh // 2
    CH = _chunks(cph)          # e.g. [512, 512, 512, 464]
    nch = len(CH)
    assert nch == 4
    nsteps = 2 * tt
    nt0 = NT0                  # leading token tiles split mini+rest
    # extra accum cols: trailing halves (banks 2-3) of the last four
    # steps at nsteps..nsteps+3, minis after them
    ncols = nsteps + 4 + nt0

    nc = bacc.Bacc("TRN2", target_bir_lowering=False, debug=False)

    h0_d = nc.dram_tensor("h0", [128, kt, tb * 128], mdt,
                          kind="ExternalInput")
    ht_d = nc.dram_tensor("ht", [ntb - 1, 128, kt, tb * 128], mdt,
                          kind="ExternalInput")
    w_d = {}
    for half in range(2):
        for ci, w in enumerate(CH):
            w_d[half, ci] = nc.dram_tensor(
                f"w_{half}_{ci}", [128, kt, w], mdt, kind="ExternalInput"
            )
    hsums_d = nc.dram_tensor("hsums", [128, ncols], f32,
                             kind="ExternalOutput")

    with tile.TileContext(nc) as tc:
        with (
            tc.tile_pool(name="w", bufs=1) as wpool,
            tc.tile_pool(name="h", bufs=1) as hpool,
            tc.tile_pool(name="stat", bufs=1) as spool,
            tc.tile_pool(name="sink", bufs=4) as kpool,
            tc.tile_pool(name="ps", bufs=2, space="PSUM") as ppool,
        ):
            wt = {}
            ht = [None] * ntb

            # First-needed tiles on the two hardware DGE queues (one
            # each, full-tile descriptors for 4KB-run packet rate);
            # everything else on the gpsimd software DGE queue, in
            # consumption order.
            h0 = hpool.tile([128, kt, tb * 128], mdt, tag="h0")
            ht[0] = h0
            nc.sync.dma_start(h0[:], h0_d[:])
            w00 = wpool.tile([128, kt, CH[0]], mdt, tag="w0_0")
            wt[0, 0] = w00
            nc.scalar.dma_start(w00[:], w_d[0, 0][:])
            for ci in range(1, nch):
                tl = wpool.tile([128, kt, CH[ci]], mdt, tag=f"w0_{ci}")
                wt[0, ci] = tl
                nc.gpsimd.dma_start(tl[:], w_d[0, ci][:])
            for b in range(1, ntb):
                tl = hpool.tile([128, kt, tb * 128], mdt, tag=f"h{b}")
                nc.gpsimd.dma_start(tl[:], ht_d[b - 1])
                ht[b] = tl
            for ci in range(nch):
                tl = wpool.tile([128, kt, CH[ci]], mdt, tag=f"w1_{ci}")
                wt[1, ci] = tl
                nc.gpsimd.dma_start(tl[:], w_d[1, ci][:])

            # PE warmup during the DMA wait: junk matmuls from a memset
            # tile (memset on the otherwise-idle vector engine) start
            # the HAM activity window so the clock boost is granted
            # (~4.3us after first activity) right as the first real
            # data lands. The window must stay gap-free until then.
            warm = kpool.tile([128, 256], mdt, tag="warm")
            nc.vector.memset(warm[:], 0.0)
            ps_w = ppool.tile([128, nch, 512], f32, tag="ps")
            for _ in range(warm_n):
                nc.tensor.matmul(
                    ps_w[:, 0, 0:128], warm[:, 0:128], warm[:, 128:256],
                    start=True, stop=True,
                )

            def mm(ps, hblk, mlo, half, ki, ci):
                rhs_t = wt[half, ci]
                w = CH[ci]
                if fp8dr:
                    nc.tensor.matmul(
                        ps[:, ci, 0:w],
                        hblk[:, 2 * ki:2 * ki + 2, mlo:mlo + 128],
                        rhs_t[:, 2 * ki:2 * ki + 2, :],
                        start=(ki == 0),
                        stop=(ki == nk - 1),
                        perf_mode=mybir.MatmulPerfMode.DoubleRow,
                    )
                else:
                    nc.tensor.matmul(
                        ps[:, ci, 0:w],
                        hblk[:, ki, mlo:mlo + 128],
                        rhs_t[:, ki, :],
                        start=(ki == 0),
                        stop=(ki == nk - 1),
                    )

            hsums = spool.tile([128, ncols], f32, tag="hsums")

            def act(ps, c0, c1, col):
                # One ACT over banks [c0, c1). Unwritten PSUM cols (the
                # tail of the last bank) read as zero, contributing
                # exp(0)=1 each; host subtracts them.
                esink = kpool.tile([128, nch * 512], bf16, tag="esink")
                nc.scalar.activation(
                    esink[:, c0 * 512:c1 * 512],
                    ps[:, c0:c1, :],
                    AF.Exp,
                    scale=act_scale,
                    accum_out=hsums[:, col:col + 1],
                )

            def step(half, t_i, order, c0=0, col=None, xcol=None):
                # xcol: split the step's ACT in two (banks [0,2) -> col,
                # banks [2,nch) -> xcol), with the first ACT issued as
                # soon as banks 0-1 are complete ("c" order) so the PSUM
                # buffer is released early enough for the tile two steps
                # ahead (a single 4-bank ACT holds it ~2.7us, longer
                # than the 1.73us the next-next allocation can wait).
                if col is None:
                    col = half * tt + t_i
                hblk = ht[t_i // tb]
                mlo = (t_i % tb) * 128
                ps = ppool.tile([128, nch, 512], f32, tag="ps")
                if order == "k":
                    for ki in range(nk):
                        for ci in range(c0, nch):
                            mm(ps, hblk, mlo, half, ki, ci)
                else:
                    for ci in range(c0, nch):
                        for ki in range(nk):
                            mm(ps, hblk, mlo, half, ki, ci)
                        if xcol is not None and ci == 1:
                            act(ps, 0, 2, col)
                if xcol is not None:
                    act(ps, 2, nch, xcol)
                else:
                    act(ps, c0, nch, col)

            # Leading token tiles: one "mini block" first — token tiles
            # 0..nt0-1 against chunk 0 only, one PSUM bank per tile,
            # k-outer. Then chunks 1-3 per tile as their DMAs land.
            # Keeps the PE dense through the DMA-paced window so the
            # HAM grant holds.
            def mini_mm(ps, bank, t_i, ki):
                mlo = (t_i % tb) * 128
                if fp8dr:
                    nc.tensor.matmul(
                        ps[:, bank, 0:CH[0]],
                        h0[:, 2 * ki:2 * ki + 2, mlo:mlo + 128],
                        w00[:, 2 * ki:2 * ki + 2, :],
                        start=(ki == 0),
                        stop=(ki == nk - 1),
                        perf_mode=mybir.MatmulPerfMode.DoubleRow,
                    )
                else:
                    nc.tensor.matmul(
                        ps[:, bank, 0:CH[0]],
                        h0[:, ki, mlo:mlo + 128],
                        w00[:, ki, :],
                        start=(ki == 0),
                        stop=(ki == nk - 1),
                    )

            # Two 2-tile mini allocations so the first pair's ACTs run
            # under the second pair's matmuls (no whole-block ACT chain
            # for the following rest-step to wait on).
            for pair in range(nt0 // 2):
                psm = ppool.tile([128, nch, 512], f32, tag="ps")
                for ki in range(nk):
                    for j in range(2):
                        mini_mm(psm, j, 2 * pair + j, ki)
                for j in range(2):
                    act(psm, j, j + 1, nsteps + 4 + 2 * pair + j)
            for t_i in range(nt0):
                step(0, t_i, "c", c0=1)
            for t_i in range(nt0, tt):
                step(0, t_i, "k")
            nc.sync.dma_start(hsums_d[:, 0:tt], hsums[:, 0:tt])
            nc.sync.dma_start(
                hsums_d[:, nsteps + 4:], hsums[:, nsteps + 4:]
            )
            for t_i in range(tt - 4):
                step(1, t_i, "k")
            nc.sync.dma_start(
                hsums_d[:, tt:nsteps - 4], hsums[:, tt:nsteps - 4]
            )
            # Two split-ACT "c" steps ahead of the split pair, so every
            # PSUM buffer from here on is released by a narrow early ACT.
            for t_i in (tt - 4, tt - 3):
                step(1, t_i, "c", xcol=nsteps + (tt - 1 - t_i))
            # Last two steps split across both PSUM buffers so the ACT
            # pipeline drains with ~1us half-width ACTs and only one
            # such ACT follows the final matmul.
            for t_i in (tt - 2, tt - 1):
                hblk = ht[t_i // tb]
                mlo = (t_i % tb) * 128
                xcol = nsteps + (tt - 1 - t_i)
                psa = ppool.tile([128, nch, 512], f32, tag="ps")
                for ci in range(2):
                    for ki in range(nk):
                        mm(psa, hblk, mlo, 1, ki, ci)
                act(psa, 0, 2, tt + t_i)
                psb = ppool.tile([128, nch, 512], f32, tag="ps")
                for ci in range(2, nch):
                    for ki in range(nk):
                        mm(psb, hblk, mlo, 1, ki, ci)
                act(psb, 2, nch, xcol)
            # Final drain from the scalar queue: it follows the last
            # accum on the same queue, so no cross-engine semaphore hop
            # sits between the last ACT and the DMA issue.
            nc.scalar.dma_start(
                hsums_d[:, nsteps - 4:nsteps + 4],
                hsums[:, nsteps - 4:nsteps + 4],
            )

    if do_compile:
        nc.compile()
    return nc


def _get_nc(kt, mode, warm_n=WARM_N):
    key = (kt, mode, warm_n)
    if key not in _CACHE:
        _CACHE[key] = _build(kt, mode, warm_n=warm_n)
    return _CACHE[key]


def kernel(hidden_states, head_weight, head_bias, labels, loss_weight):
    from concourse.bass_utils import run_bass_kernel_spmd

    bf16 = ml_dtypes.bfloat16
    fp8 = ml_dtypes.float8_e4m3
    h = np.ascontiguousarray(np.asarray(hidden_states, dtype=np.float32))
    W = np.ascontiguousarray(np.asarray(head_weight, dtype=np.float32))
    b = np.asarray(head_bias, dtype=np.float32)
    lab = np.asarray(labels).astype(np.int64)
    lw = np.asarray(loss_weight, dtype=np.float32)

    use_bias = bool(np.any(b))
    mode = "fp8dr" if (USE_FP8 and not use_bias) else "bf16"
    mdt = fp8 if mode == "fp8dr" else bf16
    wscale = W_SCALE if mode == "fp8dr" else 1.0
    kt = 9 if use_bias else 8
    nc = _get_nc(kt, mode)
    CH = _chunks(CPH)
    nsteps = 2 * TT
    nxs = 4                    # steps with split trailing-bank accum cols

    # hT[k, p, t] = h[t, k*128+p]; ht blocks [ntb, 128, kt, TBC].
    hT = np.zeros((kt, 128, T), dtype=np.float32)
    hT[:8] = np.ascontiguousarray(h.T).reshape(8, 128, T)
    if use_bias:
        hT[8, 0, :] = 1.0
    ht_blocks = np.ascontiguousarray(
        hT.reshape(kt, 128, NTB, TBC).transpose(2, 1, 0, 3).astype(mdt)
    )
    h0_block = np.ascontiguousarray(ht_blocks[0])
    ht_rest = np.ascontiguousarray(ht_blocks[1:])

    # Target logit on the host (exact f64): tgt[t] = h[t] . W[label_t]
    tgt = np.einsum(
        "td,td->t", h.astype(np.float64), W[lab].astype(np.float64)
    ) + b[lab]

    in_maps = []
    for c in range(NCORES):
        Wc = np.ascontiguousarray(W[c * VSH:(c + 1) * VSH].T) * wscale
        # wT[k, p, v] = Wc.T[k*128+p, v] (scaled)
        wT = np.zeros((kt, 128, VSH), dtype=np.float32)
        wT[:8] = Wc.reshape(8, 128, VSH)
        if use_bias:
            wT[8, 0, :] = b[c * VSH:(c + 1) * VSH]
        m = {}
        off = 0
        for half in range(2):
            for ci, w in enumerate(CH):
                blk = wT[:, :, off:off + w].transpose(1, 0, 2).astype(mdt)
                m[f"w_{half}_{ci}"] = np.ascontiguousarray(blk)
                off += w
        m["h0"] = h0_block
        m["ht"] = ht_rest
        in_maps.append(m)

    # Tile's scheduler is nondeterministic across builds and has a rare
    # dependency-emission bug: a bad roll yields a NEFF whose outputs are
    # corrupt (dropped accum slots / garbage operands). Validate against
    # hard invariants and an exact host probe; on failure, rebuild
    # (fresh schedule roll) and rerun.
    pad = len(CH) * 512 - CPH          # zero-region cols per half
    f32 = np.float32

    # One probe token per token tile, per core: replicates the device's
    # quantized math exactly (same casts) so every accum slot is checked.
    probe_p = (np.arange(TT) * 37) % 128
    probe_tok = np.arange(TT) * 128 + probe_p
    hq = h.astype(mdt).astype(f32)[probe_tok]               # [TT, D]
    if use_bias:
        hq = np.concatenate([hq, np.ones((TT, 1), f32)], axis=1)
    probe_ref = np.empty((NCORES, TT), f32)
    for c in range(NCORES):
        Wc = np.ascontiguousarray(W[c * VSH:(c + 1) * VSH]) * wscale
        Wq = Wc.astype(mdt).astype(f32)                     # [VSH, D]
        if use_bias:
            bq = b[c * VSH:(c + 1) * VSH].astype(mdt).astype(f32)
            Wq = np.concatenate([Wq, bq[:, None]], axis=1)
        lg = (hq @ Wq.T) / wscale
        probe_ref[c] = np.exp(lg).sum(axis=1)

    for attempt in range(4):
        res = run_bass_kernel_spmd(nc, in_maps, core_ids=list(range(NCORES)))

        # hsums[c][p, half*TT+t] are partial sums of exp(logit) over half
        # of core c's vocab shard for token t*128+p (+pad zero-cols).
        # Extra cols: [nsteps+j] = trailing banks (2-3) of step
        # (1, TT-1-j) for j<nxs, [nsteps+nxs+t] = chunk-0 minis of the
        # leading token tiles; fold them in.
        Sfull = np.stack([r["hsums"] for r in res.results])  # [8,128,ncols]
        Sraw = np.ascontiguousarray(Sfull[:, :, :nsteps])
        for j in range(nxs):
            Sraw[:, :, nsteps - 1 - j] += Sfull[:, :, nsteps + j]
        Sraw[:, :, :NT0] += Sfull[:, :, nsteps + nxs:]
        err_state = np.seterr(over="ignore", invalid="ignore")
        dev_probe = (
            Sraw[:, probe_p, np.arange(TT)]
            + Sraw[:, probe_p, TT + np.arange(TT)]
            - 2.0 * pad
        )                                                   # [8, TT]
        ok = (
            np.isfinite(Sfull).all()
            and (Sraw > pad).all()
            and np.allclose(dev_probe, probe_ref, rtol=5e-2, atol=1.0)
        )
        np.seterr(**err_state)
        if ok:
            break
        nc = _get_nc(kt, mode, warm_n=WARM_N + 2 * (attempt + 1))
    if not ok:
        # Every compile rolled a bad schedule: compute on host (slow but
        # exact) rather than return a corrupt result.
        logits = h @ W.T + b
        mx = logits.max(axis=1, keepdims=True)
        logz = np.log(
            np.exp((logits - mx).astype(np.float64)).sum(axis=1)
        ) + mx[:, 0]
        nll = logz - logits[np.arange(T), lab]
        lw64 = lw.astype(np.float64)
        return np.float32((lw64 * nll).sum() / lw64.sum())

    S = Sraw.reshape(NCORES, 128, 2, TT).sum(axis=2)        # [8,128,TT]
    sumexp = S.transpose(0, 2, 1).reshape(NCORES, T).astype(np.float64)
    sumexp -= 2.0 * pad
    logz = np.log(sumexp.sum(axis=0))                       # [T]

    nll = logz - tgt
    lw64 = lw.astype(np.float64)
    loss = (lw64 * nll).sum() / lw64.sum()
    return np.float32(loss)


# revision 26
# speedup vs baseline: 1.0234x; 1.0234x over previous
"""Fused linear + cross-entropy loss (chunked logsumexp) on 8 NeuronCores.

Strategy: tensor-parallel over vocab. Each core holds a 4000-row shard of
head_weight, computes logits = h @ W_c^T for all 4096 tokens (fp8e4m3
DoubleRow matmuls by default; bf16 fallback when head_bias is nonzero),
and reduces sum(exp(logit)) per token on the ACT engine (exp with
accum_out; the pre-exp rescale for the fp8 weight scaling rides the
ACT's free scale operand). The target-logit term h[t] . W[label_t] is
host glue (0.4% of the FLOPs), as are the final log over 4096 values
and the weighted mean.

Startup tuning: Tile's dependency tracking is whole-tile, so the first
matmul group waits for ALL of h block 0 + weight chunk (0,0); DMA
bandwidth scales with per-partition run length (4KB runs ~300 GB/s,
2KB ~150, 1KB ~70), so sub-tile pieces don't land any earlier than
full tiles. Hence: h0 rides the sync hardware-DGE queue (ring up
~8.2us, lands ~9.9), w00 rides the scalar hardware-DGE queue (lands
~11.0), and ALL remaining input (w0_1..3, h blocks 1-7, w1_*) rides
the gpsimd software-DGE queue, which sustains ~296 GB/s with 4KB-run
full tiles and keeps the two hardware queues free. 52 junk warmup
matmuls (from a vector-engine-memset tile) bridge the tensor queue
from preamble end (~7.1us) all the way to the first-data gate
(~13us): the HAM full-clock grant watches a free-running 4096-cycle
@1.2GHz activity window, and a >?1us PE idle gap before/near the
grant can drop the clock for 3.4-7us (measured twice; the overshoot
is cheap insurance — do not undershoot the warmup count).
The first four token tiles run as two 2-tile "mini" PSUM allocations
against chunk 0 only, followed by 3-chunk rest-steps.

Tail: the last two steps are split across both PSUM buffers so only a
~1.1us half-width ACT follows the final matmul, and hsums drains in
slices so the final output DMA is one 2KB packet. The extra
accumulator columns (minis + split-step trailing halves) are folded on
the host.
"""

import numpy as np
import ml_dtypes

T = 4096
D = 1024
V = 32000
NCORES = 8
VSH = V // NCORES        # 4000 vocab rows per core
CPH = VSH // 2           # 2000 vocab cols per half
TT = T // 128            # 32 token tiles
TBC = 512                # tokens per resident ht block
NTB = T // TBC           # 8 ht col blocks

W_SCALE = 32.0           # fp8 path: W is scaled by this before casting
USE_FP8 = True
NT0 = 4                  # leading token tiles processed as mini+rest steps
WARM_N = 52

_CACHE = {}


def _chunks(cols):
    """Split cols into matmul free-dim chunks (<=512, 16-aligned)."""
    out = []
    while cols > 0:
        c = min(cols, 512)
        out.append(c)
        cols -= c
    assert all(c % 16 == 0 for c in out)
    return out


def _build(kt, mode, t=T, vsh=VSH, d=D, warm_n=WARM_N, do_compile=True):
    """Build+compile the SPMD Bass program.

    kt: number of 128-deep k tiles (8, or 9 when a nonzero head_bias is
        folded in as an extra contraction row).
    mode: "bf16" (plain matmuls) or "fp8dr" (fp8e4m3 DoubleRow, kt even).
    """
    import concourse.bass as bass
    import concourse.mybir as mybir
    import concourse.tile as tile
    from concourse import bacc

    f32 = mybir.dt.float32
    bf16 = mybir.dt.bfloat16
    fp8 = mybir.dt.float8e4
    AF = mybir.ActivationFunctionType
    ALU = mybir.AluOpType

    fp8dr = mode == "fp8dr"
    mdt = fp8 if fp8dr else bf16
    act_scale = (1.0 / W_SCALE) if fp8dr else 1.0
    if fp8dr:
        assert kt % 2 == 0
    nk = kt // 2 if fp8dr else kt   # matmul contraction steps

    tt = t // 128
    tb = min(TBC // 128, tt)   # token tiles per ht block
    ntb = tt // tb
    cph = vsh // 2
    CH = _chunks(cph)          # e.g. [512, 512, 512, 464]
    nch = len(CH)
    assert nch == 4
    nsteps = 2 * tt
    nt0 = NT0                  # leading token tiles split mini+rest
    # extra accum cols: trailing halves of the last two steps at
    # nsteps/nsteps+1, minis after them
    ncols = nsteps + 2 + nt0

    nc = bacc.Bacc("TRN2", target_bir_lowering=False, debug=False)

    h0_d = nc.dram_tensor("h0", [128, kt, tb * 128], mdt,
                          kind="ExternalInput")
    ht_d = nc.dram_tensor("ht", [ntb - 1, 128, kt, tb * 128], mdt,
                          kind="ExternalInput")
    w_d = {}
    for half in range(2):
        for ci, w in enumerate(CH):
            w_d[half, ci] = nc.dram_tensor(
                f"w_{half}_{ci}", [128, kt, w], mdt, kind="ExternalInput"
            )
    hsums_d = nc.dram_tensor("hsums", [128, ncols], f32,
                             kind="ExternalOutput")

    with tile.TileContext(nc) as tc:
        with (
            tc.tile_pool(name="w", bufs=1) as wpool,
            tc.tile_pool(name="h", bufs=1) as hpool,
            tc.tile_pool(name="stat", bufs=1) as spool,
            tc.tile_pool(name="sink", bufs=4) as kpool,
            tc.tile_pool(name="ps", bufs=2, space="PSUM") as ppool,
        ):
            wt = {}
            ht = [None] * ntb

            # First-needed tiles on the two hardware DGE queues (one
            # each, full-tile descriptors for 4KB-run packet rate);
            # everything else on the gpsimd software DGE queue, in
            # consumption order.
            h0 = hpool.tile([128, kt, tb * 128], mdt, tag="h0")
            ht[0] = h0
            nc.sync.dma_start(h0[:], h0_d[:])
            w00 = wpool.tile([128, kt, CH[0]], mdt, tag="w0_0")
            wt[0, 0] = w00
            nc.scalar.dma_start(w00[:], w_d[0, 0][:])
            for ci in range(1, nch):
                tl = wpool.tile([128, kt, CH[ci]], mdt, tag=f"w0_{ci}")
                wt[0, ci] = tl
                nc.gpsimd.dma_start(tl[:], w_d[0, ci][:])
            for b in range(1, ntb):
                tl = hpool.tile([128, kt, tb * 128], mdt, tag=f"h{b}")
                nc.gpsimd.dma_start(tl[:], ht_d[b - 1])
                ht[b] = tl
            for ci in range(nch):
                tl = wpool.tile([128, kt, CH[ci]], mdt, tag=f"w1_{ci}")
                wt[1, ci] = tl
                nc.gpsimd.dma_start(tl[:], w_d[1, ci][:])

            # PE warmup during the DMA wait: junk matmuls from a memset
            # tile (memset on the otherwise-idle vector engine) start
            # the HAM activity window so the clock boost is granted
            # (~4.3us after first activity) right as the first real
            # data lands. The window must stay gap-free until then.
            warm = kpool.tile([128, 256], mdt, tag="warm")
            nc.vector.memset(warm[:], 0.0)
            ps_w = ppool.tile([128, nch, 512], f32, tag="ps")
            for _ in range(warm_n):
                nc.tensor.matmul(
                    ps_w[:, 0, 0:128], warm[:, 0:128], warm[:, 128:256],
                    start=True, stop=True,
                )

            def mm(ps, hblk, mlo, half, ki, ci):
                rhs_t = wt[half, ci]
                w = CH[ci]
                if fp8dr:
                    nc.tensor.matmul(
                        ps[:, ci, 0:w],
                        hblk[:, 2 * ki:2 * ki + 2, mlo:mlo + 128],
                        rhs_t[:, 2 * ki:2 * ki + 2, :],
                        start=(ki == 0),
                        stop=(ki == nk - 1),
                        perf_mode=mybir.MatmulPerfMode.DoubleRow,
                    )
                else:
                    nc.tensor.matmul(
                        ps[:, ci, 0:w],
                        hblk[:, ki, mlo:mlo + 128],
                        rhs_t[:, ki, :],
                        start=(ki == 0),
                        stop=(ki == nk - 1),
                    )

            hsums = spool.tile([128, ncols], f32, tag="hsums")

            def act(ps, c0, c1, col):
                # One ACT over banks [c0, c1). Unwritten PSUM cols (the
                # tail of the last bank) read as zero, contributing
                # exp(0)=1 each; host subtracts them.
                esink = kpool.tile([128, nch * 512], bf16, tag="esink")
                nc.scalar.activation(
                    esink[:, c0 * 512:c1 * 512],
                    ps[:, c0:c1, :],
                    AF.Exp,
                    scale=act_scale,
                    accum_out=hsums[:, col:col + 1],
                )

            # NOTE: do NOT issue an ACT over banks 0-1 of a tile while
            # later matmuls still write banks 2-3 of the same tile —
            # Tile's hazard tracking serializes the writes behind the
            # read at tile granularity (~1.8us stall each, measured).
            def step(half, t_i, order, c0=0, col=None):
                if col is None:
                    col = half * tt + t_i
                hblk = ht[t_i // tb]
                mlo = (t_i % tb) * 128
                ps = ppool.tile([128, nch, 512], f32, tag="ps")
                if order == "k":
                    for ki in range(nk):
                        for ci in range(c0, nch):
                            mm(ps, hblk, mlo, half, ki, ci)
                else:
                    for ci in range(c0, nch):
                        for ki in range(nk):
                            mm(ps, hblk, mlo, half, ki, ci)
                act(ps, c0, nch, col)

            # Leading token tiles: one "mini block" first — token tiles
            # 0..nt0-1 against chunk 0 only, one PSUM bank per tile,
            # k-outer. Then chunks 1-3 per tile as their DMAs land.
            # Keeps the PE dense through the DMA-paced window so the
            # HAM grant holds.
            def mini_mm(ps, bank, t_i, ki):
                mlo = (t_i % tb) * 128
                if fp8dr:
                    nc.tensor.matmul(
                        ps[:, bank, 0:CH[0]],
                        h0[:, 2 * ki:2 * ki + 2, mlo:mlo + 128],
                        w00[:, 2 * ki:2 * ki + 2, :],
                        start=(ki == 0),
                        stop=(ki == nk - 1),
                        perf_mode=mybir.MatmulPerfMode.DoubleRow,
                    )
                else:
                    nc.tensor.matmul(
                        ps[:, bank, 0:CH[0]],
                        h0[:, ki, mlo:mlo + 128],
                        w00[:, ki, :],
                        start=(ki == 0),
                        stop=(ki == nk - 1),
                    )

            # Two 2-tile mini allocations so the first pair's ACTs run
            # under the second pair's matmuls (no whole-block ACT chain
            # for the following rest-step to wait on).
            for pair in range(nt0 // 2):
                psm = ppool.tile([128, nch, 512], f32, tag="ps")
                for ki in range(nk):
                    for j in range(2):
                        mini_mm(psm, j, 2 * pair + j, ki)
                for j in range(2):
                    act(psm, j, j + 1, nsteps + 2 + 2 * pair + j)
            for t_i in range(nt0):
                step(0, t_i, "c", c0=1)
            for t_i in range(nt0, tt):
                step(0, t_i, "k")
            nc.sync.dma_start(hsums_d[:, 0:tt], hsums[:, 0:tt])
            nc.sync.dma_start(
                hsums_d[:, nsteps + 2:], hsums[:, nsteps + 2:]
            )
            for t_i in range(tt - 2):
                step(1, t_i, "k")
            nc.sync.dma_start(
                hsums_d[:, tt:nsteps - 2], hsums[:, tt:nsteps - 2]
            )
            # Last two steps split across both PSUM buffers so the ACT
            # pipeline drains with ~1us half-width ACTs and only one
            # such ACT follows the final matmul.
            for t_i in (tt - 2, tt - 1):
                hblk = ht[t_i // tb]
                mlo = (t_i % tb) * 128
                xcol = nsteps + (tt - 1 - t_i)
                psa = ppool.tile([128, nch, 512], f32, tag="ps")
                for ci in range(2):
                    for ki in range(nk):
                        mm(psa, hblk, mlo, 1, ki, ci)
                act(psa, 0, 2, tt + t_i)
                psb = ppool.tile([128, nch, 512], f32, tag="ps")
                for ci in range(2, nch):
                    for ki in range(nk):
                        mm(psb, hblk, mlo, 1, ki, ci)
                act(psb, 2, nch, xcol)
            nc.sync.dma_start(
                hsums_d[:, nsteps - 2:nsteps + 2],
                hsums[:, nsteps - 2:nsteps + 2],
            )

    if do_compile:
        nc.compile()
    return nc


def _get_nc(kt, mode, warm_n=WARM_N):
    key = (kt, mode, warm_n)
    if key not in _CACHE:
        _CACHE[key] = _build(kt, mode, warm_n=warm_n)
    return _CACHE[key]


def kernel(hidden_states, head_weight, head_bias, labels, loss_weight):
    from concourse.bass_utils import run_bass_kernel_spmd

    bf16 = ml_dtypes.bfloat16
    fp8 = ml_dtypes.float8_e4m3
    h = np.ascontiguousarray(np.asarray(hidden_states, dtype=np.float32))
    W = np.ascontiguousarray(np.asarray(head_weight, dtype=np.float32))
    b = np.asarray(head_bias, dtype=np.float32)
    lab = np.asarray(labels).astype(np.int64)
    lw = np.asarray(loss_weight, dtype=np.float32)

    use_bias = bool(np.any(b))
    mode = "fp8dr" if (USE_FP8 and not use_bias) else "bf16"
    mdt = fp8 if mode == "fp8dr" else bf16
    wscale = W_SCALE if mode == "fp8dr" else 1.0
    kt = 9 if use_bias else 8
    nc = _get_nc(kt, mode)
    CH = _chunks(CPH)
    nsteps = 2 * TT

    # hT[k, p, t] = h[t, k*128+p]; ht blocks [ntb, 128, kt, TBC].
    hT = np.zeros((kt, 128, T), dtype=np.float32)
    hT[:8] = np.ascontiguousarray(h.T).reshape(8, 128, T)
    if use_bias:
        hT[8, 0, :] = 1.0
    ht_blocks = np.ascontiguousarray(
        hT.reshape(kt, 128, NTB, TBC).transpose(2, 1, 0, 3).astype(mdt)
    )
    h0_block = np.ascontiguousarray(ht_blocks[0])
    ht_rest = np.ascontiguousarray(ht_blocks[1:])

    # Target logit on the host (exact f64): tgt[t] = h[t] . W[label_t]
    tgt = np.einsum(
        "td,td->t", h.astype(np.float64), W[lab].astype(np.float64)
    ) + b[lab]

    in_maps = []
    for c in range(NCORES):
        Wc = np.ascontiguousarray(W[c * VSH:(c + 1) * VSH].T) * wscale
        # wT[k, p, v] = Wc.T[k*128+p, v] (scaled)
        wT = np.zeros((kt, 128, VSH), dtype=np.float32)
        wT[:8] = Wc.reshape(8, 128, VSH)
        if use_bias:
            wT[8, 0, :] = b[c * VSH:(c + 1) * VSH]
        m = {}
        off = 0
        for half in range(2):
            for ci, w in enumerate(CH):
                blk = wT[:, :, off:off + w].transpose(1, 0, 2).astype(mdt)
                m[f"w_{half}_{ci}"] = np.ascontiguousarray(blk)
                off += w
        m["h0"] = h0_block
        m["ht"] = ht_rest
        in_maps.append(m)

    # Tile's scheduler is nondeterministic across builds and has a rare
    # dependency-emission bug: a bad roll yields a NEFF whose outputs are
    # corrupt (dropped accum slots / garbage operands). Validate against
    # hard invariants and an exact host probe; on failure, rebuild
    # (fresh schedule roll) and rerun.
    pad = len(CH) * 512 - CPH          # zero-region cols per half
    f32 = np.float32

    # One probe token per token tile, per core: replicates the device's
    # quantized math exactly (same casts) so every accum slot is checked.
    probe_p = (np.arange(TT) * 37) % 128
    probe_tok = np.arange(TT) * 128 + probe_p
    hq = h.astype(mdt).astype(f32)[probe_tok]               # [TT, D]
    if use_bias:
        hq = np.concatenate([hq, np.ones((TT, 1), f32)], axis=1)
    probe_ref = np.empty((NCORES, TT), f32)
    for c in range(NCORES):
        Wc = np.ascontiguousarray(W[c * VSH:(c + 1) * VSH]) * wscale
        Wq = Wc.astype(mdt).astype(f32)                     # [VSH, D]
        if use_bias:
            bq = b[c * VSH:(c + 1) * VSH].astype(mdt).astype(f32)
            Wq = np.concatenate([Wq, bq[:, None]], axis=1)
        lg = (hq @ Wq.T) / wscale
        probe_ref[c] = np.exp(lg).sum(axis=1)

    for attempt in range(4):
        res = run_bass_kernel_spmd(nc, in_maps, core_ids=list(range(NCORES)))

        # hsums[c][p, half*TT+t] are partial sums of exp(logit) over half
        # of core c's vocab shard for token t*128+p (+pad zero-cols).
        # Extra cols: [nsteps]/[nsteps+1] = trailing banks of the last
        # two steps, [nsteps+2+t] = chunk-0 minis of the leading token
        # tiles; fold them in.
        Sfull = np.stack([r["hsums"] for r in res.results])  # [8,128,ncols]
        Sraw = np.ascontiguousarray(Sfull[:, :, :nsteps])
        Sraw[:, :, nsteps - 1] += Sfull[:, :, nsteps]
        Sraw[:, :, nsteps - 2] += Sfull[:, :, nsteps + 1]
        Sraw[:, :, :NT0] += Sfull[:, :, nsteps + 2:]
        err_state = np.seterr(over="ignore", invalid="ignore")
        dev_probe = (
            Sraw[:, probe_p, np.arange(TT)]
            + Sraw[:, probe_p, TT + np.arange(TT)]
            - 2.0 * pad
        )                                                   # [8, TT]
        ok = (
            np.isfinite(Sfull).all()
            and (Sraw > pad).all()
            and np.allclose(dev_probe, probe_ref, rtol=5e-2, atol=1.0)
        )
        np.seterr(**err_state)
        if ok:
            break
        nc = _get_nc(kt, mode, warm_n=WARM_N + 2 * (attempt + 1))
    if not ok:
        # Every compile rolled a bad schedule: compute on host (slow but
        # exact) rather than return a corrupt result.
        logits = h @ W.T + b
        mx = logits.max(axis=1, keepdims=True)
        logz = np.log(
            np.exp((logits - mx).astype(np.float64)).sum(axis=1)
        ) + mx[:, 0]
        nll = logz - logits[np.arange(T), lab]
        lw64 = lw.astype(np.float64)
        return np.float32((lw64 * nll).sum() / lw64.sum())

    S = Sraw.reshape(NCORES, 128, 2, TT).sum(axis=2)        # [8,128,TT]
    sumexp = S.transpose(0, 2, 1).reshape(NCORES, T).astype(np.float64)
    sumexp -= 2.0 * pad
    logz = np.log(sumexp.sum(axis=0))                       # [T]

    nll = logz - tgt
    lw64 = lw.astype(np.float64)
    loss = (lw64 * nll).sum() / lw64.sum()
    return np.float32(loss)
